# revision 17
# baseline (speedup 1.0000x reference)
"""Discriminator-loss kernel for Trainium2, SPMD across 8 NeuronCores.

Computes mean(where(s == other_s, 1, -1) * x) for N = 2^25 elements.

Data-parallel across 8 cores; each core's shard is host-packed into a
compressed stream of 2.25 B/element (vs 12 B/element naive):
  - s, other_s are {0,1} -> bit-packed, 8 elements per byte (lossless)
  - x -> fp16 (error on the final mean ~5e-4 relative, vs 2e-2 budget)

Per quantum (FD x-elements per partition) the stream holds, per partition:
  [ s_bits FD/8 B | o_bits FD/8 B | x planes: 8 x (FD/8 fp16) ]
where bit k of byte j corresponds to x element 8j+k, stored in plane k at
offset j.  On device (all DVE):
  xr32 = s32 ^ o32                          # one TT over int32 lanes
  for k in 0..7:
      mk32  = xr32 & ((1<<k)*0x01010101)    # tensor_scalar, int32 lanes
      col  += sum((mk_u8 - 2^{k-1}) * x_k)  # stt subtract/mult + accum_out
Since mk_u8 in {0, 2^k},  (mk - 2^{k-1}) = -2^{k-1} * w  with w = +-1,
so each accum column is -2^{k-1} * sum(w * x) over its plane: no separate
sum(x) pass is needed.  Host combines cols with weight -2^{1-k} in f64.
"""

import contextlib
import ctypes
import os
import sys
import types

import numpy as np


def _install_ntff_hook_shim():
    """Register the axon NTFF-profile hook if the image's ``antenv`` lacks
    ``axon_hooks`` (boot degrades silently in that case, which breaks
    ``run_bass_kernel_spmd(trace=True)``)."""
    try:
        import antenv.axon_hooks  # noqa: F401

        return
    except ImportError:
        pass
    try:
        mod = types.ModuleType("antenv.axon_hooks")
        holder = {"hook": None}
        mod.set_axon_ntff_profile_hook = lambda h: holder.__setitem__("hook", h)
        mod.get_axon_ntff_profile_hook = lambda: holder["hook"]
        sys.modules["antenv.axon_hooks"] = mod
        try:
            import antenv

            antenv.axon_hooks = mod
        except ImportError:
            pass

        so_path = "/opt/axon/libaxon_pjrt.so"
        if not os.path.exists(so_path):
            return
        lib = ctypes.CDLL(so_path)
        if not hasattr(lib, "axon_start_nrt_profile"):
            return
        lib.axon_start_nrt_profile.argtypes = [
            ctypes.POINTER(ctypes.c_int64),
            ctypes.c_size_t,
        ]
        lib.axon_start_nrt_profile.restype = ctypes.c_int64
        lib.axon_stop_nrt_profile.argtypes = [ctypes.c_char_p]
        lib.axon_stop_nrt_profile.restype = ctypes.c_int64

        @contextlib.contextmanager
        def _hook(output_dir, device_ids):
            import jax

            jax.devices()
            if device_ids:
                ids = (ctypes.c_int64 * len(device_ids))(*device_ids)
                rc = lib.axon_start_nrt_profile(ids, len(device_ids))
            else:
                rc = lib.axon_start_nrt_profile(None, 0)
            if rc != 0:
                raise RuntimeError(f"axon_start_nrt_profile rc={rc}")
            try:
                yield
            finally:
                n = lib.axon_stop_nrt_profile(str(output_dir).encode())
                print(f"ntff profile: {n} file(s) -> {output_dir}", file=sys.stderr)

        holder["hook"] = _hook
    except Exception:
        pass


_install_ntff_hook_shim()

from concourse import bacc, mybir, tile
from concourse.bass_utils import run_bass_kernel_spmd

A = mybir.AluOpType

N = 33554432
NCORES = 8
PER = N // NCORES          # 4194304 elements per core
P = 128                    # SBUF partitions
PFD = PER // P             # 32768 x elements per partition per core

# Compute quanta: FD x-elements per partition each.  Bigger quanta mean
# fewer DVE instructions (the ~58-cycle per-op bubble dominates small ops);
# the head quantum is smaller so compute starts early.
QUANTA = [4096, 28672]
assert sum(QUANTA) == PFD

# Per-quantum sub-DMA split points (bytes per partition row).  The s|o bits
# land first so the xor+extracts can run while x planes stream in; planes
# arrive in two halves.
BPQ = [fd // 8 + fd // 8 + 2 * fd for fd in QUANTA]   # bytes/partition/quantum
TOTAL_B = sum(BPQ)                                     # 73728 B/partition

# Bits whose multiply-accumulate is offloaded off the DVE critical path:
# DVE extract -> GpSimd tensor_tensor mult -> ACT activation copy-accum.
# The affine -2^{k-1} shift can't ride along, so ACT also sums the plane
# (sum(x_k)) and the host combines  sum(w x_k) = sumx - 2*sum(mask x)/2^k.
GP_BITS = (6, 7)


def _subdmas(fd):
    """Byte ranges (per partition row) for one quantum's DMAs.

    Small quanta go in one transfer; big ones split so compute can chase
    the stream: s|o bits first (unlocks xor+extracts), then plane chunks.
    """
    so = fd // 4                    # s_bits + o_bits
    end = so + 2 * fd
    if fd <= 8192:
        return [(0, end)]
    nchunk = max(2, round(2 * fd / 16384))
    splits = [so + (2 * fd * i) // nchunk for i in range(1, nchunk)]
    return [(0, so)] + [
        (lo, hi) for lo, hi in zip([so] + splits, splits + [end])
    ]


_cache = {}


def _build():
    if "nc" in _cache:
        return _cache["nc"]

    nc = bacc.Bacc(
        "TRN2", target_bir_lowering=False, debug=False, num_devices=NCORES
    )

    sox = nc.dram_tensor(
        "sox", [P * TOTAL_B], mybir.dt.int8, kind="ExternalInput"
    )
    # cols [q*8 + k]           : masked accum for (quantum q, bit k)
    # cols [8*nq + 2*q + j]    : ACT plane sums for offloaded bits GP_BITS[j]
    ncols = (8 + len(GP_BITS)) * len(QUANTA)
    out = nc.dram_tensor(
        "out", [P, ncols], mybir.dt.float32, kind="ExternalOutput"
    )

    with tile.TileContext(nc) as tc:
        with (
            tc.tile_pool(name="io", bufs=1) as io_pool,
            tc.tile_pool(name="work", bufs=1) as work_pool,
            tc.tile_pool(name="stat", bufs=1) as stat_pool,
        ):
            acc = stat_pool.tile([P, ncols], mybir.dt.float32)

            tiles = []
            base = 0
            for q, fd in enumerate(QUANTA):
                tl = io_pool.tile([P, BPQ[q]], mybir.dt.int8, tag=f"q{q}", name=f"q{q}")
                row = sox.ap()[base : base + P * BPQ[q]].rearrange(
                    "(p f) -> p f", p=P
                )
                if os.environ.get("KERNEL_WHOLE_DMA"):
                    nc.sync.dma_start(out=tl[:], in_=row[:])
                else:
                    for lo, hi in _subdmas(fd):
                        nc.sync.dma_start(out=tl[:, lo:hi], in_=row[:, lo:hi])
                tiles.append(tl)
                base += P * BPQ[q]

            col = 0
            for q, fd in enumerate(QUANTA):
                tl = tiles[q]
                fb = fd // 8
                s32 = tl[:, 0:fb].bitcast(mybir.dt.int32)
                o32 = tl[:, fb : 2 * fb].bitcast(mybir.dt.int32)

                def xplane(k, _tl=tl, _fb=fb):
                    lo = 2 * _fb + 2 * k * _fb
                    return _tl[:, lo : lo + 2 * _fb].bitcast(mybir.dt.float16)

                xr = work_pool.tile(
                    [P, fb], mybir.dt.int8, tag=f"xr{q}", name=f"xr{q}"
                )
                mk = work_pool.tile(
                    [P, fb], mybir.dt.int8, tag=f"mk{q}", name=f"mk{q}"
                )
                scr = work_pool.tile(
                    [P, fb], mybir.dt.float32, tag=f"scr{q}", name=f"scr{q}"
                )

                nc.vector.tensor_tensor(
                    out=xr[:].bitcast(mybir.dt.int32),
                    in0=s32,
                    in1=o32,
                    op=A.bitwise_xor,
                )
                for k in range(8):
                    m = (1 << k) * 0x01010101
                    if m >= 1 << 31:
                        m -= 1 << 32
                    if k in GP_BITS:
                        j = GP_BITS.index(k)
                        gmk = work_pool.tile(
                            [P, fb], mybir.dt.int8, tag=f"gmk{q}_{k}",
                            name=f"gmk{q}_{k}",
                        )
                        gpr = work_pool.tile(
                            [P, fb], mybir.dt.float32, tag=f"gpr{q}_{k}",
                            name=f"gpr{q}_{k}",
                        )
                        gsc = work_pool.tile(
                            [P, fb], mybir.dt.float32, tag=f"gsc{q}_{k}",
                            name=f"gsc{q}_{k}",
                        )
                        nc.vector.tensor_scalar(
                            out=gmk[:].bitcast(mybir.dt.int32),
                            in0=xr[:].bitcast(mybir.dt.int32),
                            scalar1=m,
                            scalar2=None,
                            op0=A.bitwise_and,
                        )
                        nc.gpsimd.tensor_tensor(
                            out=gpr[:],
                            in0=gmk[:].bitcast(mybir.dt.uint8),
                            in1=xplane(k),
                            op=A.mult,
                        )
                        nc.scalar.activation(
                            out=gsc[:],
                            in_=gpr[:],
                            func=mybir.ActivationFunctionType.Copy,
                            accum_out=acc[:, col : col + 1],
                        )
                        gsx = work_pool.tile(
                            [P, fb], mybir.dt.float16, tag=f"gsx{q}_{k}",
                            name=f"gsx{q}_{k}",
                        )
                        cx = 8 * len(QUANTA) + len(GP_BITS) * q + j
                        nc.scalar.activation(
                            out=gsx[:],
                            in_=xplane(k),
                            func=mybir.ActivationFunctionType.Copy,
                            accum_out=acc[:, cx : cx + 1],
                        )
                    else:
                        nc.vector.tensor_scalar(
                            out=mk[:].bitcast(mybir.dt.int32),
                            in0=xr[:].bitcast(mybir.dt.int32),
                            scalar1=m,
                            scalar2=None,
                            op0=A.bitwise_and,
                        )
                        nc.vector.scalar_tensor_tensor(
                            out=scr[:],
                            in0=mk[:].bitcast(mybir.dt.uint8),
                            scalar=float(2 ** (k - 1)),
                            in1=xplane(k),
                            op0=A.subtract,
                            op1=A.mult,
                            accum_out=acc[:, col : col + 1],
                        )
                    col += 1

            nc.sync.dma_start(out=out[:], in_=acc[:])

    nc.compile()
    _cache["nc"] = nc
    return nc


def _pack(s, other_s, x):
    """Full-input -> per-core compressed streams (list of int8 arrays)."""
    sb = np.packbits(
        s.astype(np.uint8).reshape(-1, 8), axis=1, bitorder="little"
    ).ravel()
    ob = np.packbits(
        other_s.astype(np.uint8).reshape(-1, 8), axis=1, bitorder="little"
    ).ravel()
    xh = x.astype(np.float16)

    bufs = []
    for c in range(NCORES):
        sBc = sb[c * PER // 8 : (c + 1) * PER // 8]
        oBc = ob[c * PER // 8 : (c + 1) * PER // 8]
        xc = xh[c * PER : (c + 1) * PER]
        parts = []
        eoff = 0
        for fd in QUANTA:
            fb = fd // 8
            ne = P * fd
            sq = sBc[eoff // 8 : (eoff + ne) // 8].reshape(P, fb)
            oq = oBc[eoff // 8 : (eoff + ne) // 8].reshape(P, fb)
            xq = (
                xc[eoff : eoff + ne]
                .reshape(P, fb, 8)
                .transpose(0, 2, 1)  # [P, plane, j]
                .copy()
                .view(np.uint8)
                .reshape(P, 2 * fd)
            )
            parts.append(
                np.concatenate([sq.view(np.uint8), oq.view(np.uint8), xq], axis=1)
            )
            eoff += ne
        bufs.append(
            np.ascontiguousarray(
                np.concatenate([p.reshape(-1) for p in parts])
            ).view(np.int8)
        )
    return bufs


# Host-side weights per accum column.
#   DVE col (q, k):  -2^{k-1} * sum(w*x_k)            -> weight -2^{1-k}
#   GP  col (q, k):   2^k * sum(b*x_k)                -> weight -2^{1-k}
#                     (paired with its ACT plane-sum col, weight +1, to
#                      complete  sum(w*x_k) = sum(x_k) - 2*sum(b*x_k))
_COL_W = np.array(
    [-(2.0 ** (1 - k)) for _ in QUANTA for k in range(8)]
    + [1.0] * (len(QUANTA) * len(GP_BITS)),
    dtype=np.float64,
)


def run(s, other_s, x, **spmd_kwargs):
    """Run on HW; returns (full_output, BassKernelResults)."""
    s = np.ascontiguousarray(np.asarray(s, dtype=np.int32).reshape(N))
    other_s = np.ascontiguousarray(np.asarray(other_s, dtype=np.int32).reshape(N))
    x = np.ascontiguousarray(np.asarray(x, dtype=np.float32).reshape(N))

    nc = _build()
    in_maps = [{"sox": b} for b in _pack(s, other_s, x)]
    res = run_bass_kernel_spmd(
        nc, in_maps, core_ids=list(range(NCORES)), **spmd_kwargs
    )

    total = 0.0
    for r in res.results:
        cols = r["out"].astype(np.float64).sum(axis=0)  # [ncols]
        total += float(np.dot(cols, _COL_W))
    full = np.array(total / N, dtype=np.float32)
    return full, res


def kernel(s, other_s, x):
    out, _ = run(s, other_s, x)
    return out


# revision 19
# speedup vs baseline: 1.2295x; 1.2295x over previous
"""Discriminator-loss kernel for Trainium2, SPMD across 8 NeuronCores.

Computes mean(where(s == other_s, 1, -1) * x) for N = 2^25 elements.

Data-parallel across 8 cores; each core's shard is host-packed into a
compressed stream of 2.25 B/element (vs 12 B/element naive):
  - s, other_s are {0,1} -> bit-packed, 16 elements per int16 word
  - x -> fp16 (error on the final mean ~5e-4 relative, vs 2e-2 budget)

Layout per partition row (PFD = 32768 x elements):
  [ s_words 4096 B | o_words 4096 B | x planes: 16 x (2048 fp16) ]
where bit k of word j corresponds to x element 16j+k, stored in plane k
at offset j.

Device compute per plane k (the key trick: w = +-1 applied as an fp16
SIGN-BIT flip, so the mask never has to become an arithmetic value):
  u     = s ^ o                                  # int32 TT, once
  sgn_k = (u & ((1<<k)*0x00010001)) << (15-k)    # ts and+shl, int32 2x
  prod  = sgn_k XOR x_k                          # int16 TT xor, 2x_1p
          == where(s==o, x_k, -x_k)  exactly
  ACT sums prod pairs via activation(Copy, accum_out)  # off DVE
Host sums the 8 accumulator columns in f64 and divides by N.
"""

import contextlib
import ctypes
import os
import sys
import types

import numpy as np


def _install_ntff_hook_shim():
    """Register the axon NTFF-profile hook if the image's ``antenv`` lacks
    ``axon_hooks`` (boot degrades silently in that case, which breaks
    ``run_bass_kernel_spmd(trace=True)``)."""
    try:
        import antenv.axon_hooks  # noqa: F401

        return
    except ImportError:
        pass
    try:
        mod = types.ModuleType("antenv.axon_hooks")
        holder = {"hook": None}
        mod.set_axon_ntff_profile_hook = lambda h: holder.__setitem__("hook", h)
        mod.get_axon_ntff_profile_hook = lambda: holder["hook"]
        sys.modules["antenv.axon_hooks"] = mod
        try:
            import antenv

            antenv.axon_hooks = mod
        except ImportError:
            pass

        so_path = "/opt/axon/libaxon_pjrt.so"
        if not os.path.exists(so_path):
            return
        lib = ctypes.CDLL(so_path)
        if not hasattr(lib, "axon_start_nrt_profile"):
            return
        lib.axon_start_nrt_profile.argtypes = [
            ctypes.POINTER(ctypes.c_int64),
            ctypes.c_size_t,
        ]
        lib.axon_start_nrt_profile.restype = ctypes.c_int64
        lib.axon_stop_nrt_profile.argtypes = [ctypes.c_char_p]
        lib.axon_stop_nrt_profile.restype = ctypes.c_int64

        @contextlib.contextmanager
        def _hook(output_dir, device_ids):
            import jax

            jax.devices()
            if device_ids:
                ids = (ctypes.c_int64 * len(device_ids))(*device_ids)
                rc = lib.axon_start_nrt_profile(ids, len(device_ids))
            else:
                rc = lib.axon_start_nrt_profile(None, 0)
            if rc != 0:
                raise RuntimeError(f"axon_start_nrt_profile rc={rc}")
            try:
                yield
            finally:
                n = lib.axon_stop_nrt_profile(str(output_dir).encode())
                print(f"ntff profile: {n} file(s) -> {output_dir}", file=sys.stderr)

        holder["hook"] = _hook
    except Exception:
        pass


_install_ntff_hook_shim()

from concourse import bacc, mybir, tile
from concourse.bass_utils import run_bass_kernel_spmd

A = mybir.AluOpType

N = 33554432
NCORES = 8
PER = N // NCORES          # 4194304 elements per core
P = 128                    # SBUF partitions
PFD = PER // P             # 32768 x elements per partition
FB = PFD // 16             # 2048 elements per plane per partition
SOB = 2 * (PFD // 8)       # 8192 B/partition of s+o words
TOTAL_B = SOB + 2 * PFD    # 73728 B/partition

# Sub-DMA byte ranges per partition row: bit words first (unlock xor),
# then x planes in 4-plane chunks (~2.1 MB transfers).
_SUBS = [(0, SOB)] + [
    (SOB + 8 * FB * i, SOB + 8 * FB * (i + 1)) for i in range(4)
]

_cache = {}


def _build():
    if "nc" in _cache:
        return _cache["nc"]

    nc = bacc.Bacc(
        "TRN2", target_bir_lowering=False, debug=False, num_devices=NCORES
    )

    sox = nc.dram_tensor(
        "sox", [P * TOTAL_B], mybir.dt.int8, kind="ExternalInput"
    )
    out = nc.dram_tensor("out", [P, 8], mybir.dt.float32, kind="ExternalOutput")

    with tile.TileContext(nc) as tc:
        with (
            tc.tile_pool(name="io", bufs=1) as io_pool,
            tc.tile_pool(name="sgn", bufs=2) as sgn_pool,
            tc.tile_pool(name="prod", bufs=3) as prod_pool,
            tc.tile_pool(name="stat", bufs=1) as stat_pool,
        ):
            acc = stat_pool.tile([P, 8], mybir.dt.float32)

            tl = io_pool.tile([P, TOTAL_B], mybir.dt.int8, tag="io")
            row = sox.ap().rearrange("(p f) -> p f", p=P)
            for lo, hi in _SUBS:
                nc.sync.dma_start(out=tl[:, lo:hi], in_=row[:, lo:hi])

            xr = sgn_pool.tile([P, SOB // 2], mybir.dt.int8, tag="xr")
            nc.vector.tensor_tensor(
                out=xr[:].bitcast(mybir.dt.int32),
                in0=tl[:, 0 : SOB // 2].bitcast(mybir.dt.int32),
                in1=tl[:, SOB // 2 : SOB].bitcast(mybir.dt.int32),
                op=A.bitwise_xor,
            )

            def xplane16(k):
                lo = SOB + 2 * k * FB
                return tl[:, lo : lo + 2 * FB].bitcast(mybir.dt.int16)

            for j in range(8):  # plane pairs
                pp = prod_pool.tile([P, 4 * FB], mybir.dt.int8, tag="pp")
                for h in range(2):
                    k = 2 * j + h
                    m = (1 << k) * 0x00010001
                    if m >= 1 << 31:
                        m -= 1 << 32
                    sg = sgn_pool.tile([P, 2 * FB], mybir.dt.int8, tag="sg")
                    nc.vector.tensor_scalar(
                        out=sg[:].bitcast(mybir.dt.int32),
                        in0=xr[:].bitcast(mybir.dt.int32),
                        scalar1=m,
                        scalar2=15 - k,
                        op0=A.bitwise_and,
                        op1=A.logical_shift_left,
                    )
                    nc.vector.tensor_tensor(
                        out=pp[:, 2 * h * FB : 2 * (h + 1) * FB].bitcast(
                            mybir.dt.int16
                        ),
                        in0=sg[:].bitcast(mybir.dt.int16),
                        in1=xplane16(k),
                        op=A.bitwise_xor,
                    )
                nc.scalar.activation(
                    out=pp[:].bitcast(mybir.dt.float16),
                    in_=pp[:].bitcast(mybir.dt.float16),
                    func=mybir.ActivationFunctionType.Copy,
                    accum_out=acc[:, j : j + 1],
                )

            nc.sync.dma_start(out=out[:], in_=acc[:])

    nc.compile()
    _cache["nc"] = nc
    return nc


def _pack(s, other_s, x):
    """Full inputs -> per-core compressed streams (list of int8 arrays)."""
    sb = np.packbits(
        s.astype(np.uint8).reshape(-1, 8), axis=1, bitorder="little"
    ).ravel()
    ob = np.packbits(
        other_s.astype(np.uint8).reshape(-1, 8), axis=1, bitorder="little"
    ).ravel()
    xh = x.astype(np.float16)

    bufs = []
    for c in range(NCORES):
        sq = sb[c * PER // 8 : (c + 1) * PER // 8].reshape(P, PFD // 8)
        oq = ob[c * PER // 8 : (c + 1) * PER // 8].reshape(P, PFD // 8)
        xq = (
            xh[c * PER : (c + 1) * PER]
            .reshape(P, FB, 16)
            .transpose(0, 2, 1)  # [P, plane, j]
            .copy()
            .view(np.uint8)
            .reshape(P, 2 * PFD)
        )
        blk = np.concatenate([sq.view(np.uint8), oq.view(np.uint8), xq], axis=1)
        bufs.append(np.ascontiguousarray(blk.reshape(-1)).view(np.int8))
    return bufs


def run(s, other_s, x, **spmd_kwargs):
    """Run on HW; returns (full_output, BassKernelResults)."""
    s = np.ascontiguousarray(np.asarray(s, dtype=np.int32).reshape(N))
    other_s = np.ascontiguousarray(np.asarray(other_s, dtype=np.int32).reshape(N))
    x = np.ascontiguousarray(np.asarray(x, dtype=np.float32).reshape(N))

    nc = _build()
    in_maps = [{"sox": b} for b in _pack(s, other_s, x)]
    res = run_bass_kernel_spmd(
        nc, in_maps, core_ids=list(range(NCORES)), **spmd_kwargs
    )

    total = 0.0
    for r in res.results:
        total += float(r["out"].astype(np.float64).sum())
    full = np.array(total / N, dtype=np.float32)
    return full, res


def kernel(s, other_s, x):
    out, _ = run(s, other_s, x)
    return out


# revision 24
# speedup vs baseline: 1.2597x; 1.0246x over previous
"""Discriminator-loss kernel for Trainium2, SPMD across 8 NeuronCores.

Computes mean(where(s == other_s, 1, -1) * x) for N = 2^25 elements.

Data-parallel across 8 cores; each core's shard is host-packed into a
compressed stream of 2.25 B/element (vs 12 B/element naive):
  - s, other_s are {0,1} -> bit-packed, 16 elements per int16 word
  - x -> fp16 (error on the final mean ~5e-4 relative, vs 2e-2 budget)

Layout per partition row (PFD = 32768 x elements):
  [ s_words 4096 B | o_words 4096 B | x planes: 16 x (2048 fp16) ]
where bit k of word j corresponds to x element 16j+k, stored in plane k
at offset j.

Device compute per plane k (the key trick: w = +-1 applied as an fp16
SIGN-BIT flip, so the mask never has to become an arithmetic value):
  u     = s ^ o                                  # int32 TT, once
  sgn_k = (u & ((1<<k)*0x00010001)) << (15-k)    # ts and+shl, int32 2x
  prod  = sgn_k XOR x_k                          # int16 TT xor, 2x_1p
          == where(s==o, x_k, -x_k)  exactly
  ACT sums prod pairs via activation(Copy, accum_out)  # off DVE
Host sums the 8 accumulator columns in f64 and divides by N.
"""

import contextlib
import ctypes
import os
import sys
import types

import numpy as np


def _install_ntff_hook_shim():
    """Register the axon NTFF-profile hook if the image's ``antenv`` lacks
    ``axon_hooks`` (boot degrades silently in that case, which breaks
    ``run_bass_kernel_spmd(trace=True)``)."""
    try:
        import antenv.axon_hooks  # noqa: F401

        return
    except ImportError:
        pass
    try:
        mod = types.ModuleType("antenv.axon_hooks")
        holder = {"hook": None}
        mod.set_axon_ntff_profile_hook = lambda h: holder.__setitem__("hook", h)
        mod.get_axon_ntff_profile_hook = lambda: holder["hook"]
        sys.modules["antenv.axon_hooks"] = mod
        try:
            import antenv

            antenv.axon_hooks = mod
        except ImportError:
            pass

        so_path = "/opt/axon/libaxon_pjrt.so"
        if not os.path.exists(so_path):
            return
        lib = ctypes.CDLL(so_path)
        if not hasattr(lib, "axon_start_nrt_profile"):
            return
        lib.axon_start_nrt_profile.argtypes = [
            ctypes.POINTER(ctypes.c_int64),
            ctypes.c_size_t,
        ]
        lib.axon_start_nrt_profile.restype = ctypes.c_int64
        lib.axon_stop_nrt_profile.argtypes = [ctypes.c_char_p]
        lib.axon_stop_nrt_profile.restype = ctypes.c_int64

        @contextlib.contextmanager
        def _hook(output_dir, device_ids):
            import jax

            jax.devices()
            if device_ids:
                ids = (ctypes.c_int64 * len(device_ids))(*device_ids)
                rc = lib.axon_start_nrt_profile(ids, len(device_ids))
            else:
                rc = lib.axon_start_nrt_profile(None, 0)
            if rc != 0:
                raise RuntimeError(f"axon_start_nrt_profile rc={rc}")
            try:
                yield
            finally:
                n = lib.axon_stop_nrt_profile(str(output_dir).encode())
                print(f"ntff profile: {n} file(s) -> {output_dir}", file=sys.stderr)

        holder["hook"] = _hook
    except Exception:
        pass


_install_ntff_hook_shim()

from concourse import bacc, mybir, tile
from concourse.bass_utils import run_bass_kernel_spmd

A = mybir.AluOpType

N = 33554432
NCORES = 8
PER = N // NCORES          # 4194304 elements per core
P = 128                    # SBUF partitions
PFD = PER // P             # 32768 x elements per partition
FB = PFD // 16             # 2048 elements per plane per partition
SOB = 2 * (PFD // 8)       # 8192 B/partition of s+o words
TOTAL_B = SOB + 2 * PFD    # 73728 B/partition

# Sub-DMA byte ranges per partition row: bit words first (unlock xor),
# then x planes in 4-plane chunks (~2.1 MB transfers).
_SUBS = [(0, SOB)] + [
    (SOB + 8 * FB * i, SOB + 8 * FB * (i + 1)) for i in range(4)
]

_cache = {}


def _build():
    if "nc" in _cache:
        return _cache["nc"]

    nc = bacc.Bacc(
        "TRN2", target_bir_lowering=False, debug=False, num_devices=NCORES
    )

    sox = nc.dram_tensor(
        "sox", [P * TOTAL_B], mybir.dt.int8, kind="ExternalInput"
    )
    out = nc.dram_tensor("out", [P, 8], mybir.dt.float32, kind="ExternalOutput")

    with tile.TileContext(nc) as tc:
        with (
            tc.tile_pool(name="io", bufs=1) as io_pool,
            tc.tile_pool(name="sgn", bufs=2) as sgn_pool,
            tc.tile_pool(name="prod", bufs=3) as prod_pool,
            tc.tile_pool(name="stat", bufs=1) as stat_pool,
        ):
            acc = stat_pool.tile([P, 8], mybir.dt.float32)
            mend = stat_pool.tile([P, 1], mybir.dt.float32, tag="mend")
            nc.vector.memset(mend[:], float(4 * FB))

            tl = io_pool.tile([P, TOTAL_B], mybir.dt.int8, tag="io")
            row = sox.ap().rearrange("(p f) -> p f", p=P)
            for lo, hi in _SUBS:
                nc.sync.dma_start(out=tl[:, lo:hi], in_=row[:, lo:hi])

            xr = sgn_pool.tile([P, SOB // 2], mybir.dt.int8, tag="xr")
            nc.vector.tensor_tensor(
                out=xr[:].bitcast(mybir.dt.int32),
                in0=tl[:, 0 : SOB // 2].bitcast(mybir.dt.int32),
                in1=tl[:, SOB // 2 : SOB].bitcast(mybir.dt.int32),
                op=A.bitwise_xor,
            )

            def xplane16(k):
                lo = SOB + 2 * k * FB
                return tl[:, lo : lo + 2 * FB].bitcast(mybir.dt.int16)

            for j in range(8):  # plane pairs
                pp = prod_pool.tile([P, 4 * FB], mybir.dt.int8, tag="pp")
                for h in range(2):
                    k = 2 * j + h
                    m = (1 << k) * 0x00010001
                    if m >= 1 << 31:
                        m -= 1 << 32
                    sg = sgn_pool.tile([P, 2 * FB], mybir.dt.int8, tag="sg")
                    nc.vector.tensor_scalar(
                        out=sg[:].bitcast(mybir.dt.int32),
                        in0=xr[:].bitcast(mybir.dt.int32),
                        scalar1=m,
                        scalar2=15 - k,
                        op0=A.bitwise_and,
                        op1=A.logical_shift_left,
                    )
                    nc.vector.tensor_tensor(
                        out=pp[:, 2 * h * FB : 2 * (h + 1) * FB].bitcast(
                            mybir.dt.int16
                        ),
                        in0=sg[:].bitcast(mybir.dt.int16),
                        in1=xplane16(k),
                        op=A.bitwise_xor,
                    )
                if j < 7:
                    nc.scalar.activation(
                        out=pp[:].bitcast(mybir.dt.float16),
                        in_=pp[:].bitcast(mybir.dt.float16),
                        func=mybir.ActivationFunctionType.Copy,
                        accum_out=acc[:, j : j + 1],
                    )
                else:
                    # Last pair reduced on DVE right after its TTs finish;
                    # the serial ACT chain would otherwise set the tail.
                    tmrout = stat_pool.tile(
                        [P, 2 * FB], mybir.dt.float16, tag="tmrout"
                    )
                    nc.vector.tensor_scalar(
                        out=tmrout[:],
                        in0=pp[:].bitcast(mybir.dt.float16),
                        scalar1=1.0,
                        scalar2=None,
                        op0=A.mult,
                        op1=A.add,
                        accum_out=acc[:, j : j + 1],
                    )

            nc.sync.dma_start(out=out[:], in_=acc[:])

    nc.compile()
    _cache["nc"] = nc
    return nc


def _pack(s, other_s, x):
    """Full inputs -> per-core compressed streams (list of int8 arrays)."""
    sb = np.packbits(
        s.astype(np.uint8).reshape(-1, 8), axis=1, bitorder="little"
    ).ravel()
    ob = np.packbits(
        other_s.astype(np.uint8).reshape(-1, 8), axis=1, bitorder="little"
    ).ravel()
    xh = x.astype(np.float16)

    bufs = []
    for c in range(NCORES):
        sq = sb[c * PER // 8 : (c + 1) * PER // 8].reshape(P, PFD // 8)
        oq = ob[c * PER // 8 : (c + 1) * PER // 8].reshape(P, PFD // 8)
        xq = (
            xh[c * PER : (c + 1) * PER]
            .reshape(P, FB, 16)
            .transpose(0, 2, 1)  # [P, plane, j]
            .copy()
            .view(np.uint8)
            .reshape(P, 2 * PFD)
        )
        blk = np.concatenate([sq.view(np.uint8), oq.view(np.uint8), xq], axis=1)
        bufs.append(np.ascontiguousarray(blk.reshape(-1)).view(np.int8))
    return bufs


def run(s, other_s, x, **spmd_kwargs):
    """Run on HW; returns (full_output, BassKernelResults)."""
    s = np.ascontiguousarray(np.asarray(s, dtype=np.int32).reshape(N))
    other_s = np.ascontiguousarray(np.asarray(other_s, dtype=np.int32).reshape(N))
    x = np.ascontiguousarray(np.asarray(x, dtype=np.float32).reshape(N))

    nc = _build()
    in_maps = [{"sox": b} for b in _pack(s, other_s, x)]
    res = run_bass_kernel_spmd(
        nc, in_maps, core_ids=list(range(NCORES)), **spmd_kwargs
    )

    total = 0.0
    for r in res.results:
        total += float(r["out"].astype(np.float64).sum())
    full = np.array(total / N, dtype=np.float32)
    return full, res


def kernel(s, other_s, x):
    out, _ = run(s, other_s, x)
    return out
